# revision 14
# baseline (speedup 1.0000x reference)
"""3x3 same-padding conv (C_in=256, H=W=512, C_out=256) + bias on 8 trn2 cores.

Sharding: H split across 8 cores (64 output rows each, 1-row halo included in
each core's input slice on the host — no device-side halo exchange needed).

Per core the conv is computed as accumulated PE matmuls in float32r
(TF32-like, ~1e-4 rel err, full PE rate at N=512):
  out[co, y, :] = sum_{kh,kw,ci_half} W[kh,kw,ci_half,co].T @ xpad[ci_half, y+kh, kw:kw+512]
18 matmuls (3 kh x 2 ci_half x 3 kw) accumulate into one PSUM bank per
(row, co_half); ScalarE adds bias while draining PSUM -> SBUF; DMA out.

The first/last row blocks are small so the first matmul only waits on a
small input transfer (weights are split per co_half for the same reason)
and the final store is short.
"""
import numpy as np

import concourse.bacc as bacc
import concourse.mybir as mybir
import concourse.tile as tile
from concourse import bass_utils

NCORES = 8
CIN = 256
COUT = 256
H = 512
W = 512
RPC = H // NCORES          # output rows per core (64)
WPAD = W + 2               # width incl. zero pad cols
NTAPS = 36                 # 3*3 * 2 ci halves * 2 co halves weight tiles
BLOCKS = [2, 6] + [8] * 6 + [6, 2]   # row-block sizes (sum = RPC)
assert sum(BLOCKS) == RPC

_CACHED_NC = {}


def _build_nc(repeat=1, in_dt="float32r"):
    f32 = mybir.dt.float32
    f32r = getattr(mybir.dt, in_dt)
    nc = bacc.Bacc("TRN2", target_bir_lowering=False, debug=False,
                   num_devices=NCORES)

    xs_d = nc.dram_tensor("xs", [CIN, RPC + 2, WPAD], f32r, kind="ExternalInput")
    # weight layout: [ci_half partition, bo*18 + (kh*3+kw)*2 + bi, co]
    wt_d = nc.dram_tensor("wt", [128, NTAPS, 128], f32r, kind="ExternalInput")
    bias_d = nc.dram_tensor("bias", [128, 2], f32, kind="ExternalInput")
    out_d = nc.dram_tensor("out", [128, 2, RPC, W], f32, kind="ExternalOutput")
    # tiny output: fetching it forces execution completion without a bulk D2H
    done_d = nc.dram_tensor("done", [1, 1], f32, kind="ExternalOutput")

    mxb = max(BLOCKS)
    with tile.TileContext(nc) as tc:
        with (
            tc.tile_pool(name="const", bufs=1) as cpool,
            tc.tile_pool(name="xin", bufs=3) as xpool,
            tc.tile_pool(name="oout", bufs=2) as opool,
            tc.tile_pool(name="psum", bufs=8, space="PSUM") as psum,
        ):
            # x block 0 first, then weights split over 3 queues so the
            # first accumulation group's taps arrive earliest
            b0 = BLOCKS[0] + 2
            h0 = b0 // 2
            xa0 = xpool.tile([128, mxb + 2, WPAD], f32r, tag="xa")
            nc.sync.dma_start(xa0[:, 0:h0, :], xs_d[0:128, 0:h0, :])
            xb0 = xpool.tile([128, mxb + 2, WPAD], f32r, tag="xb")
            nc.sync.dma_start(xb0[:, 0:h0, :], xs_d[128:256, 0:h0, :])
            nc.sync.dma_start(xa0[:, h0:b0, :], xs_d[0:128, h0:b0, :])
            nc.sync.dma_start(xb0[:, h0:b0, :], xs_d[128:256, h0:b0, :])
            wtA_s = cpool.tile([128, 9, 128], f32r, tag="wtA")
            nc.sync.dma_start(wtA_s[:], wt_d[:, 0:9, :])
            wtB_s = cpool.tile([128, 9, 128], f32r, tag="wtB")
            nc.sync.dma_start(wtB_s[:], wt_d[:, 9:18, :])
            wt1_s = cpool.tile([128, 18, 128], f32r, tag="wt1")
            nc.sync.dma_start(wt1_s[:], wt_d[:, 18:36, :])
            bias_s = cpool.tile([128, 2], f32, tag="bias")
            nc.sync.dma_start(bias_s[:], bias_d[:])
            nc.sync.dma_start(done_d[:], bias_d[0:1, 0:1])

            def wtap(bo, j):
                if bo == 1:
                    return wt1_s[:, j, :]
                return wtA_s[:, j, :] if j < 9 else wtB_s[:, j - 9, :]

            for rep in range(repeat):
                r0 = 0
                for blk_i, rblk in enumerate(BLOCKS):
                    if rep == 0 and blk_i == 0:
                        xa, xb = xa0, xb0
                    else:
                        xa = xpool.tile([128, mxb + 2, WPAD], f32r, tag="xa")
                        nc.sync.dma_start(xa[:, 0:rblk + 2, :],
                                          xs_d[0:128, r0:r0 + rblk + 2, :])
                        xb = xpool.tile([128, mxb + 2, WPAD], f32r, tag="xb")
                        nc.sync.dma_start(xb[:, 0:rblk + 2, :],
                                          xs_d[128:256, r0:r0 + rblk + 2, :])
                    oa = opool.tile([128, mxb, W], f32, tag="oa")
                    ob = opool.tile([128, mxb, W], f32, tag="ob")
                    for yy in range(rblk):
                        for bo in range(2):
                            acc = psum.tile([128, W], f32, tag="acc")
                            k = 0
                            for kh in range(3):
                                for bi in range(2):
                                    xt = xa if bi == 0 else xb
                                    for kw in range(3):
                                        j = (kh * 3 + kw) * 2 + bi
                                        nc.tensor.matmul(
                                            acc[:],
                                            wtap(bo, j),
                                            xt[:, yy + kh, kw:kw + W],
                                            start=(k == 0),
                                            stop=(k == NTAPS // 2 - 1),
                                        )
                                        k += 1
                            ot = oa if bo == 0 else ob
                            nc.scalar.activation(
                                ot[:, yy, :], acc[:],
                                mybir.ActivationFunctionType.Identity,
                                bias=bias_s[:, bo:bo + 1],
                            )
                    nc.sync.dma_start(out_d[:, 0, r0:r0 + rblk, :],
                                      oa[:, 0:rblk, :])
                    nc.sync.dma_start(out_d[:, 1, r0:r0 + rblk, :],
                                      ob[:, 0:rblk, :])
                    r0 += rblk

    nc.compile()
    return nc


def _get_nc(repeat=1, in_dt="float32r"):
    key = (repeat, in_dt)
    if key not in _CACHED_NC:
        _CACHED_NC[key] = _build_nc(repeat, in_dt)
    return _CACHED_NC[key]


def _prep_inputs(x, W_, b, in_dt="float32r"):
    npdt = np.float32
    if in_dt == "bfloat16":
        import ml_dtypes
        npdt = ml_dtypes.bfloat16
    xs_all = np.zeros((NCORES, CIN, RPC + 2, WPAD), npdt)
    for m in range(NCORES):
        g0 = max(0, m * RPC - 1)
        g1 = min(H, m * RPC + RPC + 1)
        r0 = g0 - (m * RPC - 1)
        xs_all[m, :, r0:r0 + (g1 - g0), 1:1 + W] = x[:, g0:g1, :]
    # [kh, kw, ci, co] -> [ci_p, bo, kh, kw, bi, co_m] -> [128, 36, 128]
    wt = np.ascontiguousarray(
        W_.reshape(3, 3, 2, 128, 2, 128).transpose(3, 4, 0, 1, 2, 5)
        .reshape(128, NTAPS, 128).astype(npdt))
    bias = np.ascontiguousarray(b.reshape(2, 128).T)
    return xs_all, wt, bias


def kernel(x, W, b, _trace=False):
    x = np.asarray(x, dtype=np.float32)
    W = np.asarray(W, dtype=np.float32)
    b = np.asarray(b, dtype=np.float32)
    nc = _get_nc()
    xs_all, wt, bias = _prep_inputs(x, W, b)
    in_maps = [{"xs": xs_all[m], "wt": wt, "bias": bias} for m in range(NCORES)]
    res = bass_utils.run_bass_kernel_spmd(
        nc, in_maps, list(range(NCORES)), trace=_trace)
    arr = np.stack([res.results[m]["out"] for m in range(NCORES)], axis=0)
    # [m, p, bo, yy, x] -> [bo, p, m, yy, x] -> [C_out, H, W]
    full = arr.transpose(2, 1, 0, 3, 4).reshape(COUT, H, 512)
    if _trace:
        return full, res
    return full
